# revision 10
# baseline (speedup 1.0000x reference)
"""BiLSTM-CRF Trainium2 kernel (8 NeuronCores, SPMD).

Strategy:
- 8 cores form 4 redundant pairs. Even cores compute the forward LSTM + CRF
  forward scan; odd cores compute the backward LSTM (fed time-reversed token
  indices) and contribute the backward emission half. Direction is encoded
  purely in per-core *input data* (reversed index vectors, transposed
  weights, permutation vectors) so every core runs the identical program.
- LSTM recurrence: weight-stationary fp32 matmuls on the PE
  (64x [128k,128g] tiles per step), x-projections precomputed in bulk.
- Emission halves are exchanged within each pair via AllReduce(add) after a
  data-driven time-reversal (indirect row gather keyed by a per-core
  permutation input).
- CRF Viterbi: forward max-plus scan replicating the reference arithmetic
  (emission add fused as a second per-partition scalar), with per-step
  backpointers via max/max_index. Backtrace (pure O(L) pointer chasing, no
  FLOPs) runs on host.
"""

import numpy as np

L_FULL = 4096
V, NI = 50000, 64
DW, DI, H2, T = 512, 128, 512, 128
NG = 16  # gate tiles (2048/128)
KH = 4   # hidden k-tiles (512/128)
KX = 5   # input k-tiles (640/128)

# psum gate-column order: group-major [i_g f_g o_g g~_g for g in 0..3] so the
# elementwise epilogue of hidden-group g can overlap group g+1's weight loads
_BLOCK_START = [0, 512, 1536, 1024]  # i, f, o, g~ row-block starts in reference order
GATE_ROW_PERM = np.concatenate(
    [np.arange(_BLOCK_START[b] + 128 * g, _BLOCK_START[b] + 128 * (g + 1))
     for g in range(4) for b in range(4)])


def _tileize(mat_kd_g, kt):
    """[kt*128, G] -> host layout [128, kt*G] so sbuf[p, k, g] = mat[k*128+p, g]."""
    G = mat_kd_g.shape[1]
    return np.ascontiguousarray(
        mat_kd_g.reshape(kt, 128, G).transpose(1, 0, 2).reshape(128, kt * G))


def make_core_inputs(inputs, core_is_bwd, L=L_FULL):
    """Build the per-core input map (all float32 unless noted)."""
    d = inputs
    rev = slice(None, None, -1)
    if not core_is_bwd:
        tok = d["sentence"]; it = d["intent"]
        w_ih, w_hh, b = d["w_ih_f"], d["w_hh_f"], d["b_f"]
        h0, c0 = d["h0"][0], d["c0"][0]
        lin_half = d["lin_w"][:, :H2]
        lin_b = d["lin_b"].reshape(T, 1)
        transA = d["trans"]
        initv = d["start_trans"].reshape(T, 1)
        endv = d["end_trans"].reshape(T, 1)
        perm = np.arange(L, dtype=np.int32)
    else:
        tok = d["sentence"][rev]; it = d["intent"][rev]
        w_ih, w_hh, b = d["w_ih_b"], d["w_hh_b"], d["b_b"]
        h0, c0 = d["h0"][1], d["c0"][1]
        lin_half = d["lin_w"][:, H2:]
        lin_b = np.zeros((T, 1), np.float32)
        transA = np.ascontiguousarray(d["trans"].T)
        initv = d["end_trans"].reshape(T, 1)
        endv = d["start_trans"].reshape(T, 1)
        perm = (L - 1) - np.arange(L, dtype=np.int32)

    f32 = np.float32
    wihT = np.ascontiguousarray(w_ih[GATE_ROW_PERM].T)   # [640, 2048]
    whhT = np.ascontiguousarray(w_hh[GATE_ROW_PERM].T)   # [512, 2048]
    linT = np.ascontiguousarray(lin_half.T)              # [512, 128]
    return {
        "tok_idx": tok.astype(np.int32),
        "int_idx": it.astype(np.int32),
        "perm": perm.astype(np.int32),
        "word_emb": d["word_emb"].astype(f32),
        "intent_emb": d["intent_emb"].astype(f32),
        "wih": _tileize(wihT, KX).astype(f32),           # [128, 5*2048]
        "whh": _tileize(whhT, KH).astype(f32),           # [128, 4*2048]
        "bias": np.ascontiguousarray(
            b[GATE_ROW_PERM].reshape(NG, 128).T).astype(f32),  # [128, 16]
        "h0c0": np.concatenate(
            [h0.reshape(KH, 128).T, c0.reshape(KH, 128).T], axis=1).astype(f32),  # [128, 8]
        "linT": _tileize(linT, KH).astype(f32),          # [128, 4*128]
        "linb": lin_b.astype(f32),                       # [128, 1]
        "transA": transA.astype(f32),                    # [128, 128]
        "initv": initv.astype(f32),                      # [128, 1]
        "endv": endv.astype(f32),                        # [128, 1]
        "identity": np.eye(128, dtype=f32),              # [128, 128]
    }


def build_program(nc, tc, n_cores, L=L_FULL, U=16, debug=False):
    """Emit the full SPMD program into TileContext tc for Bass nc."""
    import concourse.bass as bass
    import concourse.mybir as mybir

    dt = mybir.dt
    AF = mybir.ActivationFunctionType
    OP = mybir.AluOpType
    NT = L // 128      # 128-token chunks
    assert L % U == 0
    NI_LSTM = L // U
    CU = max(d for d in range(1, 65) if (L - 1) % d == 0)
    NC_CRF = (L - 1) // CU
    assert NC_CRF * CU == L - 1
    NCHX = min(512, L)  # xproj/emis time-chunk

    def din(name, shape, dtype=dt.float32):
        return nc.dram_tensor(name, list(shape), dtype, kind="ExternalInput").ap()

    def dout(name, shape, dtype=dt.float32):
        return nc.dram_tensor(name, list(shape), dtype, kind="ExternalOutput").ap()

    tok = din("tok_idx", [L], dt.int32)
    iti = din("int_idx", [L], dt.int32)
    perm = din("perm", [L], dt.int32)
    wemb = din("word_emb", [V, DW])
    iemb = din("intent_emb", [NI, DI])
    wih = din("wih", [128, KX * 2048])
    whh = din("whh", [128, KH * 2048])
    bias = din("bias", [128, NG])
    h0c0 = din("h0c0", [128, 2 * KH])
    linT = din("linT", [128, KH * 128])
    linb = din("linb", [128, 1])
    transA = din("transA", [128, 128])
    initv = din("initv", [128, 1])
    endv = din("endv", [128, 1])
    ident = din("identity", [128, 128])

    hist_out = dout("hist", [128, L - 1], dt.uint32)
    score_out = dout("scorevec", [128, 1])
    dbg = {}
    if debug:
        dbg["emis"] = dout("dbg_emis", [128, L])
        dbg["hf"] = dout("dbg_h", [128, KH, L])
        dbg["a"] = dout("dbg_a", [128, NG, L])

    # DRAM intermediates
    a_dram = nc.dram_tensor("a_dram", [128, NG, L], dt.float32).ap()
    h_dram = nc.dram_tensor("h_dram", [128, KH, L], dt.float32).ap()
    tm_dram = nc.dram_tensor("tm_dram", [L, 128], dt.float32).ap()
    e_dram = nc.dram_tensor("e_dram", [128, L], dt.float32).ap()

    groups = [[2 * i, 2 * i + 1] for i in range(n_cores // 2)]

    # ---- persistent SBUF (whole kernel) ----
    from contextlib import ExitStack
    ctx = ExitStack()
    pers = ctx.enter_context(tc.tile_pool(name="pers", bufs=1))
    ident_sb = pers.tile([128, 128], dt.float32, tag="ident")
    nc.sync.dma_start(ident_sb[:], ident)

    # ================= Phase A+B: gather + transpose =================
    # x feature-major xT_sb[p, k, t] = x[t, 128k+p], k: 0-3 word, 4 intent
    xT_sb = pers.tile([128, KX, L], dt.float32, tag="xT")
    with tc.tile_pool(name="gath", bufs=3) as gp, \
         tc.tile_pool(name="gpsum", bufs=3, space="PSUM") as gpp:
        idx_sb = gp.tile([128, NT], dt.int32, tag="idx")
        nc.sync.dma_start(idx_sb[:], tok.rearrange("(c p) -> p c", p=128))
        iidx_sb = gp.tile([128, NT], dt.int32, tag="iidx")
        nc.sync.dma_start(iidx_sb[:], iti.rearrange("(c p) -> p c", p=128))
        for c in range(NT):
            wrow = gp.tile([128, DW], dt.float32, tag="wrow")
            nc.gpsimd.indirect_dma_start(
                out=wrow[:], out_offset=None, in_=wemb,
                in_offset=bass.IndirectOffsetOnAxis(ap=idx_sb[:, c:c + 1], axis=0))
            irow = gp.tile([128, DI], dt.float32, tag="irow")
            nc.gpsimd.indirect_dma_start(
                out=irow[:], out_offset=None, in_=iemb,
                in_offset=bass.IndirectOffsetOnAxis(ap=iidx_sb[:, c:c + 1], axis=0))
            for k in range(KX):
                pt = gpp.tile([128, 128], dt.float32, tag="pt")
                src = wrow[:, 128 * k:128 * (k + 1)] if k < 4 else irow[:]
                nc.tensor.transpose(pt[:], src, ident_sb[:])
                nc.vector.tensor_copy(xT_sb[:, k, 128 * c:128 * (c + 1)], pt[:])

    # ================= Phase C: x-projection =================
    with tc.tile_pool(name="xw", bufs=1) as xwp, \
         tc.tile_pool(name="xo", bufs=3) as xop, \
         tc.tile_pool(name="xpsum", bufs=2, space="PSUM") as xpp:
        wih_sb = xwp.tile([128, KX, 2048], dt.float32, tag="wih")
        nc.sync.dma_start(wih_sb[:], wih.rearrange("p (k g) -> p k g", k=KX))
        bias_sb = xwp.tile([128, NG], dt.float32, tag="bias")
        nc.sync.dma_start(bias_sb[:], bias)
        for gt in range(NG):
            for ch in range(L // NCHX):
                ps = xpp.tile([128, NCHX], dt.float32, tag="xp")
                for k in range(KX):
                    nc.tensor.matmul(
                        ps[:], lhsT=wih_sb[:, k, 128 * gt:128 * (gt + 1)],
                        rhs=xT_sb[:, k, NCHX * ch:NCHX * (ch + 1)],
                        start=(k == 0), stop=(k == KX - 1))
                ao = xop.tile([128, NCHX], dt.float32, tag="ao")
                nc.vector.tensor_scalar_add(ao[:], ps[:], bias_sb[:, gt:gt + 1])
                nc.sync.dma_start(a_dram[:, gt, NCHX * ch:NCHX * (ch + 1)], ao[:])
    if debug:
        with tc.tile_pool(name="dbga", bufs=2) as dp:
            t_ = dp.tile([128, NG * L], dt.float32, tag="dbga")
            nc.sync.dma_start(t_[:], a_dram.rearrange("p g t -> p (g t)"))
            nc.sync.dma_start(dbg["a"].rearrange("p g t -> p (g t)"), t_[:])

    # ================= Phase D: LSTM recurrence =================
    with tc.tile_pool(name="lw", bufs=1) as lwp, \
         tc.tile_pool(name="lst", bufs=1) as lsp, \
         tc.tile_pool(name="lio", bufs=2) as liop, \
         tc.tile_pool(name="ltmp", bufs=2) as ltp, \
         tc.tile_pool(name="lpsum", bufs=2, space="PSUM") as lpp:
        whh_sb = lwp.tile([128, KH, 2048], dt.float32, tag="whh")
        nc.sync.dma_start(whh_sb[:], whh.rearrange("p (k g) -> p k g", k=KH))
        h_state = lsp.tile([128, KH], dt.float32, tag="hstate")
        c_state = lsp.tile([128, KH], dt.float32, tag="cstate")
        h0c0_sb = lsp.tile([128, 2 * KH], dt.float32, tag="h0c0")
        nc.sync.dma_start(h0c0_sb[:], h0c0)
        nc.vector.tensor_copy(h_state[:], h0c0_sb[:, 0:KH])
        nc.vector.tensor_copy(c_state[:], h0c0_sb[:, KH:2 * KH])

        with tc.For_i(0, NI_LSTM, 1, hint_engines=(mybir.EngineType.PE,)) as ci:
            a_sb = liop.tile([128, NG, U], dt.float32, tag="a_sb")
            nc.sync.dma_start(a_sb[:], a_dram[:, :, bass.ts(ci, U)])
            # one h tile per hidden group so group deps stay fine-grained
            hsv = [liop.tile([128, U], dt.float32, tag=f"hs{g}", name=f"hs{g}")
                   for g in range(KH)]
            for u in range(U):
                gp_ = lpp.tile([128, NG], dt.float32, tag="gates")
                for g in range(KH):
                    for ty in range(4):
                        col = 4 * g + ty
                        for k in range(KH):
                            rhs = (h_state[:, k:k + 1] if u == 0
                                   else hsv[k][:, u - 1:u])
                            nc.tensor.matmul(
                                gp_[:, col:col + 1],
                                lhsT=whh_sb[:, k, 128 * col:128 * (col + 1)],
                                rhs=rhs, start=(k == 0), stop=(k == KH - 1))
                    # epilogue for group g (cols 4g..4g+3 = i,f,o,g~)
                    gs = ltp.tile([128, 4], dt.float32, tag="gs")
                    nc.vector.tensor_add(gs[:], gp_[:, 4 * g:4 * g + 4],
                                         a_sb[:, 4 * g:4 * g + 4, u])
                    nc.scalar.activation(gs[:, 0:3], gs[:, 0:3], AF.Sigmoid)
                    nc.scalar.activation(gs[:, 3:4], gs[:, 3:4], AF.Tanh)
                    t1 = ltp.tile([128, 1], dt.float32, tag="t1")
                    nc.vector.tensor_mul(t1[:], gs[:, 1:2], c_state[:, g:g + 1])
                    t2 = ltp.tile([128, 1], dt.float32, tag="t2")
                    nc.vector.tensor_mul(t2[:], gs[:, 0:1], gs[:, 3:4])
                    nc.vector.tensor_add(c_state[:, g:g + 1], t1[:], t2[:])
                    tc_ = ltp.tile([128, 1], dt.float32, tag="tc")
                    nc.scalar.activation(tc_[:], c_state[:, g:g + 1], AF.Tanh)
                    nc.vector.tensor_mul(hsv[g][:, u:u + 1], gs[:, 2:3], tc_[:])
            for g in range(KH):
                nc.vector.tensor_copy(h_state[:, g:g + 1], hsv[g][:, U - 1:U])
                nc.sync.dma_start(h_dram[:, g, bass.ts(ci, U)], hsv[g][:])
    if debug:
        with tc.tile_pool(name="dbgh", bufs=2) as dp:
            t_ = dp.tile([128, KH * L], dt.float32, tag="dbgh")
            nc.sync.dma_start(t_[:], h_dram.rearrange("p k t -> p (k t)"))
            nc.sync.dma_start(dbg["hf"].rearrange("p k t -> p (k t)"), t_[:])

    # ================= Phase E: emission half =================
    emis_sb = pers.tile([128, L], dt.float32, tag="emis")
    with tc.tile_pool(name="ew", bufs=1) as ewp, \
         tc.tile_pool(name="eio", bufs=2) as eiop, \
         tc.tile_pool(name="epsum", bufs=2, space="PSUM") as epp:
        lin_sb = ewp.tile([128, KH, 128], dt.float32, tag="lin")
        nc.sync.dma_start(lin_sb[:], linT.rearrange("p (k m) -> p k m", k=KH))
        linb_sb = ewp.tile([128, 1], dt.float32, tag="linb")
        nc.sync.dma_start(linb_sb[:], linb)
        for ch in range(L // NCHX):
            hch = eiop.tile([128, KH, NCHX], dt.float32, tag="hch")
            nc.sync.dma_start(hch[:], h_dram[:, :, NCHX * ch:NCHX * (ch + 1)])
            ps = epp.tile([128, NCHX], dt.float32, tag="ep")
            for k in range(KH):
                nc.tensor.matmul(ps[:], lhsT=lin_sb[:, k, :],
                                 rhs=hch[:, k, :],
                                 start=(k == 0), stop=(k == KH - 1))
            nc.vector.tensor_scalar_add(
                emis_sb[:, NCHX * ch:NCHX * (ch + 1)], ps[:], linb_sb[:, 0:1])

    # ================= Phase F: exchange (reverse via perm + AllReduce) ====
    with tc.tile_pool(name="fx", bufs=2) as fxp, \
         tc.tile_pool(name="fpsum", bufs=2, space="PSUM") as fpp, \
         tc.tile_pool(name="fdram", bufs=1, space="DRAM") as fdp:
        tm_sb = fxp.tile([128, NT, 128], dt.float32, tag="tm")
        for c in range(NT):
            pt = fpp.tile([128, 128], dt.float32, tag="fpt")
            nc.tensor.transpose(pt[:], emis_sb[:, 128 * c:128 * (c + 1)], ident_sb[:])
            nc.vector.tensor_copy(tm_sb[:, c, :], pt[:])
        nc.sync.dma_start(tm_dram.rearrange("(c p) f -> p c f", p=128), tm_sb[:])
        perm_sb = fxp.tile([128, NT], dt.int32, tag="perm")
        nc.sync.dma_start(perm_sb[:], perm.rearrange("(c p) -> p c", p=128))
        g_sb = fxp.tile([128, NT, 128], dt.float32, tag="g_sb")
        for c in range(NT):
            nc.gpsimd.indirect_dma_start(
                out=g_sb[:, c, :], out_offset=None, in_=tm_dram,
                in_offset=bass.IndirectOffsetOnAxis(ap=perm_sb[:, c:c + 1], axis=0))
        contrib = fdp.tile([L, 128], dt.float32, tag="contrib")
        enat = fdp.tile([L, 128], dt.float32, tag="enat")
        nc.sync.dma_start(contrib[:].rearrange("(c p) f -> p c f", p=128), g_sb[:])
        nc.gpsimd.collective_compute(
            "AllReduce", mybir.AluOpType.add, replica_groups=groups,
            ins=[contrib[:]], outs=[enat[:]])
        g2_sb = fxp.tile([128, NT, 128], dt.float32, tag="g2")
        nc.sync.dma_start(g2_sb[:], enat[:].rearrange("(c p) f -> p c f", p=128))
        for c in range(NT):
            pt = fpp.tile([128, 128], dt.float32, tag="fpt")
            nc.tensor.transpose(pt[:], g2_sb[:, c, :], ident_sb[:])
            nc.vector.tensor_copy(emis_sb[:, 128 * c:128 * (c + 1)], pt[:])
        nc.sync.dma_start(e_dram[:], emis_sb[:])
    if debug:
        with tc.tile_pool(name="dbge", bufs=2) as dp:
            nc.sync.dma_start(dbg["emis"], emis_sb[:])

    # ================= Phase G: CRF forward scan =================
    with tc.tile_pool(name="cw", bufs=1) as cwp, \
         tc.tile_pool(name="cio", bufs=2) as ciop, \
         tc.tile_pool(name="ctmp", bufs=3) as ctp, \
         tc.tile_pool(name="cpsum", bufs=2, space="PSUM") as cpp:
        trans_sb = cwp.tile([128, 128], dt.float32, tag="trans")
        nc.sync.dma_start(trans_sb[:], transA)
        s_state = cwp.tile([128, 1], dt.float32, tag="sstate")  # pre-emission S
        initv_sb = cwp.tile([128, 1], dt.float32, tag="initv")
        nc.sync.dma_start(initv_sb[:], initv)
        nc.vector.tensor_copy(s_state[:], initv_sb[:])

        with tc.For_i(0, NC_CRF, 1,
                      hint_engines=(mybir.EngineType.PE, mybir.EngineType.DVE)) as ci:
            ech = ciop.tile([128, CU], dt.float32, tag="ech")
            nc.sync.dma_start(ech[:], e_dram[:, bass.ts(ci, CU)])
            hch = ciop.tile([128, CU], dt.uint32, tag="hch")
            m8p = None
            for u in range(CU):
                # step t = ci*CU+u+1: tmp[i,j] = (trans[i,j] + S_{t-1}[i]) + e_{t-1}[i]
                tmp = ctp.tile([128, 128], dt.float32, tag="ctmp")
                s_ap = s_state[:, 0:1] if u == 0 else m8p[:, 0:1]
                nc.vector.tensor_scalar(
                    tmp[:], trans_sb[:], s_ap, ech[:, u:u + 1],
                    op0=OP.add, op1=OP.add)
                pt = cpp.tile([128, 128], dt.float32, tag="cpt")
                nc.tensor.transpose(pt[:], tmp[:], ident_sb[:])
                m8 = ctp.tile([128, 8], dt.float32, tag="m8")
                nc.vector.max(out=m8[:], in_=pt[:])
                h8 = ctp.tile([128, 8], dt.uint32, tag="h8")
                nc.vector.max_index(out=h8[:], in_max=m8[:], in_values=pt[:])
                nc.vector.tensor_copy(hch[:, u:u + 1], h8[:, 0:1])
                m8p = m8
            nc.vector.tensor_copy(s_state[:], m8p[:, 0:1])
            nc.sync.dma_start(hist_out[:, bass.ts(ci, CU)], hch[:])

        # scorevec = (S_last + e_{L-1}) + endv
        endv_sb = cwp.tile([128, 1], dt.float32, tag="endv")
        nc.sync.dma_start(endv_sb[:], endv)
        sc = cwp.tile([128, 1], dt.float32, tag="sc")
        nc.vector.tensor_add(sc[:], s_state[:], emis_sb[:, L - 1:L])
        nc.vector.tensor_add(sc[:], sc[:], endv_sb[:])
        nc.sync.dma_start(score_out, sc[:])

    ctx.close()
    return dbg


def backtrace(hist, scorevec, L):
    """hist: [128, L-1] (col s = ref hist[s]); scorevec: [128]."""
    last = int(np.argmax(scorevec))
    best = np.float32(scorevec[last])
    path = np.empty(L, np.int32)
    path[L - 1] = last
    cur = last
    for t in range(L - 2, -1, -1):
        cur = int(hist[cur, t])
        path[t] = cur
    return path, best


def kernel(**inputs):
    import concourse.bacc as bacc
    import concourse.tile as tile
    from concourse.bass_utils import run_bass_kernel_spmd

    inputs = {k: np.asarray(v) for k, v in inputs.items()}
    n_cores = 8
    nc = bacc.Bacc("TRN2", target_bir_lowering=False, debug=False,
                   num_devices=n_cores)
    with tile.TileContext(nc) as tc:
        build_program(nc, tc, n_cores, L=L_FULL, U=16)
    nc.compile()

    in_fwd = make_core_inputs(inputs, core_is_bwd=False)
    in_bwd = make_core_inputs(inputs, core_is_bwd=True)
    in_maps = [in_fwd if c % 2 == 0 else in_bwd for c in range(n_cores)]
    res = run_bass_kernel_spmd(nc, in_maps, list(range(n_cores)))
    r0 = res.results[0]
    path, best = backtrace(r0["hist"], r0["scorevec"].reshape(-1), L_FULL)
    return path, best


# revision 14
# speedup vs baseline: 1.0320x; 1.0320x over previous
"""BiLSTM-CRF Trainium2 kernel (8 NeuronCores, SPMD).

Strategy:
- 8 cores form 4 redundant pairs. Even cores compute the forward LSTM + CRF
  forward scan; odd cores compute the backward LSTM (fed time-reversed token
  indices) and contribute the backward emission half. Direction is encoded
  purely in per-core *input data* (reversed index vectors, transposed
  weights, permutation vectors) so every core runs the identical program.
- LSTM recurrence: weight-stationary fp32 matmuls on the PE
  (64x [128k,128g] tiles per step), x-projections precomputed in bulk.
- Emission halves are exchanged within each pair via AllReduce(add) after a
  data-driven time-reversal (indirect row gather keyed by a per-core
  permutation input).
- CRF Viterbi: forward max-plus scan replicating the reference arithmetic
  (emission add fused as a second per-partition scalar), with per-step
  backpointers via max/max_index. Backtrace (pure O(L) pointer chasing, no
  FLOPs) runs on host.
"""

import numpy as np

L_FULL = 4096
V, NI = 50000, 64
DW, DI, H2, T = 512, 128, 512, 128
NG = 16  # gate tiles (2048/128)
KH = 4   # hidden k-tiles (512/128)
KX = 5   # input k-tiles (640/128)

# psum gate-column order: group-major [i_g f_g o_g g~_g for g in 0..3] so the
# elementwise epilogue of hidden-group g can overlap group g+1's weight loads
_BLOCK_START = [0, 512, 1536, 1024]  # i, f, o, g~ row-block starts in reference order
GATE_ROW_PERM = np.concatenate(
    [np.arange(_BLOCK_START[b] + 128 * g, _BLOCK_START[b] + 128 * (g + 1))
     for g in range(4) for b in range(4)])


def _tileize(mat_kd_g, kt):
    """[kt*128, G] -> host layout [128, kt*G] so sbuf[p, k, g] = mat[k*128+p, g]."""
    G = mat_kd_g.shape[1]
    return np.ascontiguousarray(
        mat_kd_g.reshape(kt, 128, G).transpose(1, 0, 2).reshape(128, kt * G))


def make_core_inputs(inputs, core_is_bwd, L=L_FULL):
    """Build the per-core input map (all float32 unless noted)."""
    d = inputs
    rev = slice(None, None, -1)
    if not core_is_bwd:
        tok = d["sentence"]; it = d["intent"]
        w_ih, w_hh, b = d["w_ih_f"], d["w_hh_f"], d["b_f"]
        h0, c0 = d["h0"][0], d["c0"][0]
        lin_half = d["lin_w"][:, :H2]
        lin_b = d["lin_b"].reshape(T, 1)
        transA = d["trans"]
        initv = d["start_trans"].reshape(T, 1)
        endv = d["end_trans"].reshape(T, 1)
        perm = np.arange(L, dtype=np.int32)
    else:
        tok = d["sentence"][rev]; it = d["intent"][rev]
        w_ih, w_hh, b = d["w_ih_b"], d["w_hh_b"], d["b_b"]
        h0, c0 = d["h0"][1], d["c0"][1]
        lin_half = d["lin_w"][:, H2:]
        lin_b = np.zeros((T, 1), np.float32)
        transA = np.ascontiguousarray(d["trans"].T)
        initv = d["end_trans"].reshape(T, 1)
        endv = d["start_trans"].reshape(T, 1)
        perm = (L - 1) - np.arange(L, dtype=np.int32)

    f32 = np.float32
    wihT = np.ascontiguousarray(w_ih[GATE_ROW_PERM].T)   # [640, 2048]
    whhT = np.ascontiguousarray(w_hh[GATE_ROW_PERM].T)   # [512, 2048]
    linT = np.ascontiguousarray(lin_half.T)              # [512, 128]
    return {
        "tok_idx": tok.astype(np.int32),
        "int_idx": it.astype(np.int32),
        "perm": perm.astype(np.int32),
        "word_emb": d["word_emb"].astype(f32),
        "intent_emb": d["intent_emb"].astype(f32),
        "wih": _tileize(wihT, KX).astype(f32),           # [128, 5*2048]
        "whh": _tileize(whhT, KH).astype(f32),           # [128, 4*2048]
        "bias": np.ascontiguousarray(
            b[GATE_ROW_PERM].reshape(NG, 128).T).astype(f32),  # [128, 16]
        "h0c0": np.concatenate(
            [h0.reshape(KH, 128).T, c0.reshape(KH, 128).T], axis=1).astype(f32),  # [128, 8]
        "linT": _tileize(linT, KH).astype(f32),          # [128, 4*128]
        "linb": lin_b.astype(f32),                       # [128, 1]
        "transA": transA.astype(f32),                    # [128, 128]
        "initv": initv.astype(f32),                      # [128, 1]
        "endv": endv.astype(f32),                        # [128, 1]
        "identity": np.eye(128, dtype=f32),              # [128, 128]
    }


def build_program(nc, tc, n_cores, L=L_FULL, U=16, debug=False,
                  skip_lstm=False, skip_crf=False, stag=False):
    """Emit the full SPMD program into TileContext tc for Bass nc."""
    import concourse.bass as bass
    import concourse.mybir as mybir

    dt = mybir.dt
    AF = mybir.ActivationFunctionType
    OP = mybir.AluOpType
    NT = L // 128      # 128-token chunks
    assert L % U == 0
    NI_LSTM = L // U
    CU = max(d for d in range(1, 65) if (L - 1) % d == 0)
    NC_CRF = (L - 1) // CU
    assert NC_CRF * CU == L - 1
    NCHX = min(512, L)  # xproj/emis time-chunk

    def din(name, shape, dtype=dt.float32):
        return nc.dram_tensor(name, list(shape), dtype, kind="ExternalInput").ap()

    def dout(name, shape, dtype=dt.float32):
        return nc.dram_tensor(name, list(shape), dtype, kind="ExternalOutput").ap()

    tok = din("tok_idx", [L], dt.int32)
    iti = din("int_idx", [L], dt.int32)
    perm = din("perm", [L], dt.int32)
    wemb = din("word_emb", [V, DW])
    iemb = din("intent_emb", [NI, DI])
    wih = din("wih", [128, KX * 2048])
    whh = din("whh", [128, KH * 2048])
    bias = din("bias", [128, NG])
    h0c0 = din("h0c0", [128, 2 * KH])
    linT = din("linT", [128, KH * 128])
    linb = din("linb", [128, 1])
    transA = din("transA", [128, 128])
    initv = din("initv", [128, 1])
    endv = din("endv", [128, 1])
    ident = din("identity", [128, 128])

    hist_out = dout("hist", [128, L - 1], dt.uint32)
    score_out = dout("scorevec", [128, 1])
    dbg = {}
    if debug:
        dbg["emis"] = dout("dbg_emis", [128, L])
        dbg["hf"] = dout("dbg_h", [128, KH, L])
        dbg["a"] = dout("dbg_a", [128, NG, L])

    # DRAM intermediates
    a_dram = nc.dram_tensor("a_dram", [128, NG, L], dt.float32).ap()
    h_dram = nc.dram_tensor("h_dram", [128, KH, L], dt.float32).ap()
    tm_dram = nc.dram_tensor("tm_dram", [L, 128], dt.float32).ap()
    e_dram = nc.dram_tensor("e_dram", [128, L], dt.float32).ap()

    groups = [[2 * i, 2 * i + 1] for i in range(n_cores // 2)]

    # ---- persistent SBUF (whole kernel) ----
    from contextlib import ExitStack
    ctx = ExitStack()
    pers = ctx.enter_context(tc.tile_pool(name="pers", bufs=1))
    ident_sb = pers.tile([128, 128], dt.float32, tag="ident")
    nc.sync.dma_start(ident_sb[:], ident)

    # ================= Phase A+B: gather + transpose =================
    # x feature-major xT_sb[p, k, t] = x[t, 128k+p], k: 0-3 word, 4 intent
    xT_sb = pers.tile([128, KX, L], dt.float32, tag="xT")
    with tc.tile_pool(name="gath", bufs=3) as gp, \
         tc.tile_pool(name="gpsum", bufs=3, space="PSUM") as gpp:
        idx_sb = gp.tile([128, NT], dt.int32, tag="idx")
        nc.sync.dma_start(idx_sb[:], tok.rearrange("(c p) -> p c", p=128))
        iidx_sb = gp.tile([128, NT], dt.int32, tag="iidx")
        nc.sync.dma_start(iidx_sb[:], iti.rearrange("(c p) -> p c", p=128))
        for c in range(NT):
            wrow = gp.tile([128, DW], dt.float32, tag="wrow")
            nc.gpsimd.indirect_dma_start(
                out=wrow[:], out_offset=None, in_=wemb,
                in_offset=bass.IndirectOffsetOnAxis(ap=idx_sb[:, c:c + 1], axis=0))
            irow = gp.tile([128, DI], dt.float32, tag="irow")
            nc.gpsimd.indirect_dma_start(
                out=irow[:], out_offset=None, in_=iemb,
                in_offset=bass.IndirectOffsetOnAxis(ap=iidx_sb[:, c:c + 1], axis=0))
            for k in range(KX):
                pt = gpp.tile([128, 128], dt.float32, tag="pt")
                src = wrow[:, 128 * k:128 * (k + 1)] if k < 4 else irow[:]
                nc.tensor.transpose(pt[:], src, ident_sb[:])
                nc.vector.tensor_copy(xT_sb[:, k, 128 * c:128 * (c + 1)], pt[:])

    # ================= Phase C: x-projection =================
    with tc.tile_pool(name="xw", bufs=1) as xwp, \
         tc.tile_pool(name="xo", bufs=3) as xop, \
         tc.tile_pool(name="xpsum", bufs=2, space="PSUM") as xpp:
        wih_sb = xwp.tile([128, KX, 2048], dt.float32, tag="wih")
        nc.sync.dma_start(wih_sb[:], wih.rearrange("p (k g) -> p k g", k=KX))
        bias_sb = xwp.tile([128, NG], dt.float32, tag="bias")
        nc.sync.dma_start(bias_sb[:], bias)
        for gt in range(NG):
            for ch in range(L // NCHX):
                ps = xpp.tile([128, NCHX], dt.float32, tag="xp")
                for k in range(KX):
                    nc.tensor.matmul(
                        ps[:], lhsT=wih_sb[:, k, 128 * gt:128 * (gt + 1)],
                        rhs=xT_sb[:, k, NCHX * ch:NCHX * (ch + 1)],
                        start=(k == 0), stop=(k == KX - 1))
                ao = xop.tile([128, NCHX], dt.float32, tag="ao")
                nc.vector.tensor_scalar_add(ao[:], ps[:], bias_sb[:, gt:gt + 1])
                nc.sync.dma_start(a_dram[:, gt, NCHX * ch:NCHX * (ch + 1)], ao[:])
    if debug:
        with tc.tile_pool(name="dbga", bufs=2) as dp:
            t_ = dp.tile([128, NG * L], dt.float32, tag="dbga")
            nc.sync.dma_start(t_[:], a_dram.rearrange("p g t -> p (g t)"))
            nc.sync.dma_start(dbg["a"].rearrange("p g t -> p (g t)"), t_[:])

    # ================= Phase D: LSTM recurrence =================
    if skip_lstm:
        NI_LSTM = 1
    with tc.tile_pool(name="lw", bufs=1) as lwp, \
         tc.tile_pool(name="lst", bufs=1) as lsp, \
         tc.tile_pool(name="lio", bufs=2) as liop, \
         tc.tile_pool(name="ltmp", bufs=2) as ltp, \
         tc.tile_pool(name="lpsum", bufs=2, space="PSUM") as lpp:
        whh_sb = lwp.tile([128, KH, 2048], dt.float32, tag="whh")
        nc.sync.dma_start(whh_sb[:], whh.rearrange("p (k g) -> p k g", k=KH))
        h_state = lsp.tile([128, KH], dt.float32, tag="hstate")
        c_state = lsp.tile([128, KH], dt.float32, tag="cstate")
        h0c0_sb = lsp.tile([128, 2 * KH], dt.float32, tag="h0c0")
        nc.sync.dma_start(h0c0_sb[:], h0c0)
        nc.vector.tensor_copy(h_state[:], h0c0_sb[:, 0:KH])
        nc.vector.tensor_copy(c_state[:], h0c0_sb[:, KH:2 * KH])

        with tc.For_i(0, NI_LSTM, 1, hint_engines=(mybir.EngineType.PE,),
                      staggered_reset=stag) as ci:
            a_sb = liop.tile([128, NG, U], dt.float32, tag="a_sb")
            nc.sync.dma_start(a_sb[:], a_dram[:, :, bass.ts(ci, U)])
            # one h tile per hidden group so group deps stay fine-grained
            hsv = [liop.tile([128, U], dt.float32, tag=f"hs{g}", name=f"hs{g}")
                   for g in range(KH)]
            for u in range(U):
                gp_ = lpp.tile([128, NG], dt.float32, tag="gates")
                for g in range(KH):
                    for ty in range(4):
                        col = 4 * g + ty
                        for k in range(KH):
                            rhs = (h_state[:, k:k + 1] if u == 0
                                   else hsv[k][:, u - 1:u])
                            nc.tensor.matmul(
                                gp_[:, col:col + 1],
                                lhsT=whh_sb[:, k, 128 * col:128 * (col + 1)],
                                rhs=rhs, start=(k == 0), stop=(k == KH - 1))
                    # epilogue for group g (cols 4g..4g+3 = i,f,o,g~)
                    gs = ltp.tile([128, 4], dt.float32, tag="gs")
                    nc.vector.tensor_add(gs[:], gp_[:, 4 * g:4 * g + 4],
                                         a_sb[:, 4 * g:4 * g + 4, u])
                    nc.scalar.activation(gs[:, 0:3], gs[:, 0:3], AF.Sigmoid)
                    nc.scalar.activation(gs[:, 3:4], gs[:, 3:4], AF.Tanh)
                    t1 = ltp.tile([128, 1], dt.float32, tag="t1")
                    nc.vector.tensor_mul(t1[:], gs[:, 1:2], c_state[:, g:g + 1])
                    t2 = ltp.tile([128, 1], dt.float32, tag="t2")
                    nc.vector.tensor_mul(t2[:], gs[:, 0:1], gs[:, 3:4])
                    nc.vector.tensor_add(c_state[:, g:g + 1], t1[:], t2[:])
                    tc_ = ltp.tile([128, 1], dt.float32, tag="tc")
                    nc.scalar.activation(tc_[:], c_state[:, g:g + 1], AF.Tanh)
                    nc.vector.tensor_mul(hsv[g][:, u:u + 1], gs[:, 2:3], tc_[:])
            for g in range(KH):
                nc.vector.tensor_copy(h_state[:, g:g + 1], hsv[g][:, U - 1:U])
                nc.sync.dma_start(h_dram[:, g, bass.ts(ci, U)], hsv[g][:])
    if debug:
        with tc.tile_pool(name="dbgh", bufs=2) as dp:
            t_ = dp.tile([128, KH * L], dt.float32, tag="dbgh")
            nc.sync.dma_start(t_[:], h_dram.rearrange("p k t -> p (k t)"))
            nc.sync.dma_start(dbg["hf"].rearrange("p k t -> p (k t)"), t_[:])

    # ================= Phase E: emission half =================
    emis_sb = pers.tile([128, L], dt.float32, tag="emis")
    with tc.tile_pool(name="ew", bufs=1) as ewp, \
         tc.tile_pool(name="eio", bufs=2) as eiop, \
         tc.tile_pool(name="epsum", bufs=2, space="PSUM") as epp:
        lin_sb = ewp.tile([128, KH, 128], dt.float32, tag="lin")
        nc.sync.dma_start(lin_sb[:], linT.rearrange("p (k m) -> p k m", k=KH))
        linb_sb = ewp.tile([128, 1], dt.float32, tag="linb")
        nc.sync.dma_start(linb_sb[:], linb)
        for ch in range(L // NCHX):
            hch = eiop.tile([128, KH, NCHX], dt.float32, tag="hch")
            nc.sync.dma_start(hch[:], h_dram[:, :, NCHX * ch:NCHX * (ch + 1)])
            ps = epp.tile([128, NCHX], dt.float32, tag="ep")
            for k in range(KH):
                nc.tensor.matmul(ps[:], lhsT=lin_sb[:, k, :],
                                 rhs=hch[:, k, :],
                                 start=(k == 0), stop=(k == KH - 1))
            nc.vector.tensor_scalar_add(
                emis_sb[:, NCHX * ch:NCHX * (ch + 1)], ps[:], linb_sb[:, 0:1])

    # ================= Phase F: exchange (reverse via perm + AllReduce) ====
    with tc.tile_pool(name="fx", bufs=2) as fxp, \
         tc.tile_pool(name="fpsum", bufs=2, space="PSUM") as fpp, \
         tc.tile_pool(name="fdram", bufs=1, space="DRAM") as fdp:
        tm_sb = fxp.tile([128, NT, 128], dt.float32, tag="tm")
        for c in range(NT):
            pt = fpp.tile([128, 128], dt.float32, tag="fpt")
            nc.tensor.transpose(pt[:], emis_sb[:, 128 * c:128 * (c + 1)], ident_sb[:])
            nc.vector.tensor_copy(tm_sb[:, c, :], pt[:])
        nc.sync.dma_start(tm_dram.rearrange("(c p) f -> p c f", p=128), tm_sb[:])
        perm_sb = fxp.tile([128, NT], dt.int32, tag="perm")
        nc.sync.dma_start(perm_sb[:], perm.rearrange("(c p) -> p c", p=128))
        g_sb = fxp.tile([128, NT, 128], dt.float32, tag="g_sb")
        for c in range(NT):
            nc.gpsimd.indirect_dma_start(
                out=g_sb[:, c, :], out_offset=None, in_=tm_dram,
                in_offset=bass.IndirectOffsetOnAxis(ap=perm_sb[:, c:c + 1], axis=0))
        contrib = fdp.tile([L, 128], dt.float32, tag="contrib")
        enat = fdp.tile([L, 128], dt.float32, tag="enat")
        nc.sync.dma_start(contrib[:].rearrange("(c p) f -> p c f", p=128), g_sb[:])
        nc.gpsimd.collective_compute(
            "AllReduce", mybir.AluOpType.add, replica_groups=groups,
            ins=[contrib[:]], outs=[enat[:]])
        g2_sb = fxp.tile([128, NT, 128], dt.float32, tag="g2")
        nc.sync.dma_start(g2_sb[:], enat[:].rearrange("(c p) f -> p c f", p=128))
        for c in range(NT):
            pt = fpp.tile([128, 128], dt.float32, tag="fpt")
            nc.tensor.transpose(pt[:], g2_sb[:, c, :], ident_sb[:])
            nc.vector.tensor_copy(emis_sb[:, 128 * c:128 * (c + 1)], pt[:])
        nc.sync.dma_start(e_dram[:], emis_sb[:])
    if debug:
        with tc.tile_pool(name="dbge", bufs=2) as dp:
            nc.sync.dma_start(dbg["emis"], emis_sb[:])

    # ================= Phase G: CRF forward scan =================
    with tc.tile_pool(name="cw", bufs=1) as cwp, \
         tc.tile_pool(name="cio", bufs=2) as ciop, \
         tc.tile_pool(name="ctmp", bufs=3) as ctp, \
         tc.tile_pool(name="cpsum", bufs=2, space="PSUM") as cpp:
        trans_sb = cwp.tile([128, 128], dt.float32, tag="trans")
        nc.sync.dma_start(trans_sb[:], transA)
        s_state = cwp.tile([128, 1], dt.float32, tag="sstate")  # pre-emission S
        initv_sb = cwp.tile([128, 1], dt.float32, tag="initv")
        nc.sync.dma_start(initv_sb[:], initv)
        nc.vector.tensor_copy(s_state[:], initv_sb[:])

        if skip_crf:
            NC_CRF = 1
        with tc.For_i(0, NC_CRF, 1,
                      hint_engines=(mybir.EngineType.PE, mybir.EngineType.DVE),
                      staggered_reset=stag) as ci:
            ech = ciop.tile([128, CU], dt.float32, tag="ech")
            nc.sync.dma_start(ech[:], e_dram[:, bass.ts(ci, CU)])
            hch = ciop.tile([128, CU], dt.uint32, tag="hch")
            m8p = None
            for u in range(CU):
                # step t = ci*CU+u+1: tmp[i,j] = (trans[i,j] + S_{t-1}[i]) + e_{t-1}[i]
                tmp = ctp.tile([128, 128], dt.float32, tag="ctmp")
                s_ap = s_state[:, 0:1] if u == 0 else m8p[:, 0:1]
                nc.vector.tensor_scalar(
                    tmp[:], trans_sb[:], s_ap, ech[:, u:u + 1],
                    op0=OP.add, op1=OP.add)
                pt = cpp.tile([128, 128], dt.float32, tag="cpt")
                nc.tensor.transpose(pt[:], tmp[:], ident_sb[:])
                m8 = ctp.tile([128, 8], dt.float32, tag="m8")
                nc.vector.max(out=m8[:], in_=pt[:])
                h8 = ctp.tile([128, 8], dt.uint32, tag="h8")
                nc.vector.max_index(out=h8[:], in_max=m8[:], in_values=pt[:])
                nc.vector.tensor_copy(hch[:, u:u + 1], h8[:, 0:1])
                m8p = m8
            nc.vector.tensor_copy(s_state[:], m8p[:, 0:1])
            nc.sync.dma_start(hist_out[:, bass.ts(ci, CU)], hch[:])

        # scorevec = (S_last + e_{L-1}) + endv
        endv_sb = cwp.tile([128, 1], dt.float32, tag="endv")
        nc.sync.dma_start(endv_sb[:], endv)
        sc = cwp.tile([128, 1], dt.float32, tag="sc")
        nc.vector.tensor_add(sc[:], s_state[:], emis_sb[:, L - 1:L])
        nc.vector.tensor_add(sc[:], sc[:], endv_sb[:])
        nc.sync.dma_start(score_out, sc[:])

    ctx.close()
    return dbg


def backtrace(hist, scorevec, L):
    """hist: [128, L-1] (col s = ref hist[s]); scorevec: [128]."""
    last = int(np.argmax(scorevec))
    best = np.float32(scorevec[last])
    path = np.empty(L, np.int32)
    path[L - 1] = last
    cur = last
    for t in range(L - 2, -1, -1):
        cur = int(hist[cur, t])
        path[t] = cur
    return path, best


def kernel(**inputs):
    import concourse.bacc as bacc
    import concourse.tile as tile
    from concourse.bass_utils import run_bass_kernel_spmd

    inputs = {k: np.asarray(v) for k, v in inputs.items()}
    n_cores = 8
    nc = bacc.Bacc("TRN2", target_bir_lowering=False, debug=False,
                   num_devices=n_cores)
    with tile.TileContext(nc) as tc:
        build_program(nc, tc, n_cores, L=L_FULL, U=16)
    nc.compile()

    in_fwd = make_core_inputs(inputs, core_is_bwd=False)
    in_bwd = make_core_inputs(inputs, core_is_bwd=True)
    in_maps = [in_fwd if c % 2 == 0 else in_bwd for c in range(n_cores)]
    res = run_bass_kernel_spmd(nc, in_maps, list(range(n_cores)))
    r0 = res.results[0]
    path, best = backtrace(r0["hist"], r0["scorevec"].reshape(-1), L_FULL)
    return path, best


# revision 15
# speedup vs baseline: 1.6326x; 1.5820x over previous
"""BiLSTM-CRF Trainium2 kernel (8 NeuronCores, SPMD).

Strategy:
- 8 cores form 4 redundant pairs. Even cores compute the forward LSTM + CRF
  forward scan; odd cores compute the backward LSTM (fed time-reversed token
  indices) and contribute the backward emission half. Direction is encoded
  purely in per-core *input data* (reversed index vectors, transposed
  weights, permutation vectors) so every core runs the identical program.
- LSTM recurrence: weight-stationary fp32 matmuls on the PE
  (64x [128k,128g] tiles per step), x-projections precomputed in bulk.
- Emission halves are exchanged within each pair via AllReduce(add) after a
  data-driven time-reversal (indirect row gather keyed by a per-core
  permutation input).
- CRF Viterbi: forward max-plus scan replicating the reference arithmetic
  (emission add fused as a second per-partition scalar), with per-step
  backpointers via max/max_index. Backtrace (pure O(L) pointer chasing, no
  FLOPs) runs on host.
"""

import numpy as np

L_FULL = 4096
V, NI = 50000, 64
DW, DI, H2, T = 512, 128, 512, 128
NG = 16  # gate tiles (2048/128)
KH = 4   # hidden k-tiles (512/128)
KX = 5   # input k-tiles (640/128)

# psum gate-column order: group-major [i_g f_g o_g g~_g for g in 0..3] so the
# elementwise epilogue of hidden-group g can overlap group g+1's weight loads
_BLOCK_START = [0, 512, 1536, 1024]  # i, f, o, g~ row-block starts in reference order
GATE_ROW_PERM = np.concatenate(
    [np.arange(_BLOCK_START[b] + 128 * g, _BLOCK_START[b] + 128 * (g + 1))
     for g in range(4) for b in range(4)])


def _tileize(mat_kd_g, kt):
    """[kt*128, G] -> host layout [128, kt*G] so sbuf[p, k, g] = mat[k*128+p, g]."""
    G = mat_kd_g.shape[1]
    return np.ascontiguousarray(
        mat_kd_g.reshape(kt, 128, G).transpose(1, 0, 2).reshape(128, kt * G))


def make_core_inputs(inputs, core_is_bwd, L=L_FULL):
    """Build the per-core input map (all float32 unless noted)."""
    d = inputs
    rev = slice(None, None, -1)
    if not core_is_bwd:
        tok = d["sentence"]; it = d["intent"]
        w_ih, w_hh, b = d["w_ih_f"], d["w_hh_f"], d["b_f"]
        h0, c0 = d["h0"][0], d["c0"][0]
        lin_half = d["lin_w"][:, :H2]
        lin_b = d["lin_b"].reshape(T, 1)
        transA = d["trans"]
        initv = d["start_trans"].reshape(T, 1)
        endv = d["end_trans"].reshape(T, 1)
        perm = np.arange(L, dtype=np.int32)
    else:
        tok = d["sentence"][rev]; it = d["intent"][rev]
        w_ih, w_hh, b = d["w_ih_b"], d["w_hh_b"], d["b_b"]
        h0, c0 = d["h0"][1], d["c0"][1]
        lin_half = d["lin_w"][:, H2:]
        lin_b = np.zeros((T, 1), np.float32)
        transA = np.ascontiguousarray(d["trans"].T)
        initv = d["end_trans"].reshape(T, 1)
        endv = d["start_trans"].reshape(T, 1)
        perm = (L - 1) - np.arange(L, dtype=np.int32)

    f32 = np.float32
    wihT = np.ascontiguousarray(w_ih[GATE_ROW_PERM].T)   # [640, 2048]
    whhT = np.ascontiguousarray(w_hh[GATE_ROW_PERM].T)   # [512, 2048]
    linT = np.ascontiguousarray(lin_half.T)              # [512, 128]
    return {
        "tok_idx": tok.astype(np.int32),
        "int_idx": it.astype(np.int32),
        "perm": perm.astype(np.int32),
        "word_emb": d["word_emb"].astype(f32),
        "intent_emb": d["intent_emb"].astype(f32),
        "wih": _tileize(wihT, KX).astype(f32),           # [128, 5*2048]
        "whh": _tileize(whhT, KH).astype(f32),           # [128, 4*2048]
        "bias": np.ascontiguousarray(
            b[GATE_ROW_PERM].reshape(NG, 128).T).astype(f32),  # [128, 16]
        "h0c0": np.concatenate(
            [h0.reshape(KH, 128).T, c0.reshape(KH, 128).T], axis=1).astype(f32),  # [128, 8]
        "linT": _tileize(linT, KH).astype(f32),          # [128, 4*128]
        "linb": lin_b.astype(f32),                       # [128, 1]
        "transA": transA.astype(f32),                    # [128, 128]
        "initv": initv.astype(f32),                      # [128, 1]
        "endv": endv.astype(f32),                        # [128, 1]
        "identity": np.eye(128, dtype=f32),              # [128, 128]
    }


def build_program(nc, tc, n_cores, L=L_FULL, U=16, debug=False,
                  skip_lstm=False, skip_crf=False, stag=False):
    """Emit the full SPMD program into TileContext tc for Bass nc."""
    import concourse.bass as bass
    import concourse.mybir as mybir

    dt = mybir.dt
    AF = mybir.ActivationFunctionType
    OP = mybir.AluOpType
    NT = L // 128      # 128-token chunks
    assert L % U == 0
    NI_LSTM = L // U
    CU = max(d for d in range(1, 65) if (L - 1) % d == 0)
    NC_CRF = (L - 1) // CU
    assert NC_CRF * CU == L - 1
    NCHX = min(512, L)  # xproj/emis time-chunk

    def din(name, shape, dtype=dt.float32):
        return nc.dram_tensor(name, list(shape), dtype, kind="ExternalInput").ap()

    def dout(name, shape, dtype=dt.float32):
        return nc.dram_tensor(name, list(shape), dtype, kind="ExternalOutput").ap()

    tok = din("tok_idx", [L], dt.int32)
    iti = din("int_idx", [L], dt.int32)
    perm = din("perm", [L], dt.int32)
    wemb = din("word_emb", [V, DW])
    iemb = din("intent_emb", [NI, DI])
    wih = din("wih", [128, KX * 2048])
    whh = din("whh", [128, KH * 2048])
    bias = din("bias", [128, NG])
    h0c0 = din("h0c0", [128, 2 * KH])
    linT = din("linT", [128, KH * 128])
    linb = din("linb", [128, 1])
    transA = din("transA", [128, 128])
    initv = din("initv", [128, 1])
    endv = din("endv", [128, 1])
    ident = din("identity", [128, 128])

    hist_out = dout("hist", [128, L - 1], dt.uint32)
    score_out = dout("scorevec", [128, 1])
    dbg = {}
    if debug:
        dbg["emis"] = dout("dbg_emis", [128, L])
        dbg["hf"] = dout("dbg_h", [128, KH, L])
        dbg["a"] = dout("dbg_a", [128, NG, L])

    # DRAM intermediates
    a_dram = nc.dram_tensor("a_dram", [128, NG, L], dt.float32).ap()
    h_dram = nc.dram_tensor("h_dram", [128, KH, L], dt.float32).ap()
    tm_dram = nc.dram_tensor("tm_dram", [L, 128], dt.float32).ap()
    e_dram = nc.dram_tensor("e_dram", [128, L], dt.float32).ap()

    groups = [[2 * i, 2 * i + 1] for i in range(n_cores // 2)]

    # ---- persistent SBUF (whole kernel) ----
    from contextlib import ExitStack
    ctx = ExitStack()
    pers = ctx.enter_context(tc.tile_pool(name="pers", bufs=1))
    ident_sb = pers.tile([128, 128], dt.float32, tag="ident")
    nc.sync.dma_start(ident_sb[:], ident)

    # ================= Phase A+B: gather + transpose =================
    # x feature-major xT_sb[p, k, t] = x[t, 128k+p], k: 0-3 word, 4 intent
    xT_sb = pers.tile([128, KX, L], dt.float32, tag="xT")
    with tc.tile_pool(name="gath", bufs=3) as gp, \
         tc.tile_pool(name="gpsum", bufs=3, space="PSUM") as gpp:
        idx_sb = gp.tile([128, NT], dt.int32, tag="idx")
        nc.sync.dma_start(idx_sb[:], tok.rearrange("(c p) -> p c", p=128))
        iidx_sb = gp.tile([128, NT], dt.int32, tag="iidx")
        nc.sync.dma_start(iidx_sb[:], iti.rearrange("(c p) -> p c", p=128))
        for c in range(NT):
            wrow = gp.tile([128, DW], dt.float32, tag="wrow")
            nc.gpsimd.indirect_dma_start(
                out=wrow[:], out_offset=None, in_=wemb,
                in_offset=bass.IndirectOffsetOnAxis(ap=idx_sb[:, c:c + 1], axis=0))
            irow = gp.tile([128, DI], dt.float32, tag="irow")
            nc.gpsimd.indirect_dma_start(
                out=irow[:], out_offset=None, in_=iemb,
                in_offset=bass.IndirectOffsetOnAxis(ap=iidx_sb[:, c:c + 1], axis=0))
            for k in range(KX):
                pt = gpp.tile([128, 128], dt.float32, tag="pt")
                src = wrow[:, 128 * k:128 * (k + 1)] if k < 4 else irow[:]
                nc.tensor.transpose(pt[:], src, ident_sb[:])
                nc.vector.tensor_copy(xT_sb[:, k, 128 * c:128 * (c + 1)], pt[:])

    # ================= Phase C: x-projection =================
    with tc.tile_pool(name="xw", bufs=1) as xwp, \
         tc.tile_pool(name="xo", bufs=3) as xop, \
         tc.tile_pool(name="xpsum", bufs=2, space="PSUM") as xpp:
        wih_sb = xwp.tile([128, KX, 2048], dt.float32, tag="wih")
        nc.sync.dma_start(wih_sb[:], wih.rearrange("p (k g) -> p k g", k=KX))
        bias_sb = xwp.tile([128, NG], dt.float32, tag="bias")
        nc.sync.dma_start(bias_sb[:], bias)
        for gt in range(NG):
            for ch in range(L // NCHX):
                ps = xpp.tile([128, NCHX], dt.float32, tag="xp")
                for k in range(KX):
                    nc.tensor.matmul(
                        ps[:], lhsT=wih_sb[:, k, 128 * gt:128 * (gt + 1)],
                        rhs=xT_sb[:, k, NCHX * ch:NCHX * (ch + 1)],
                        start=(k == 0), stop=(k == KX - 1))
                ao = xop.tile([128, NCHX], dt.float32, tag="ao")
                nc.vector.tensor_scalar_add(ao[:], ps[:], bias_sb[:, gt:gt + 1])
                nc.sync.dma_start(a_dram[:, gt, NCHX * ch:NCHX * (ch + 1)], ao[:])
    if debug:
        with tc.tile_pool(name="dbga", bufs=2) as dp:
            t_ = dp.tile([128, NG * L], dt.float32, tag="dbga")
            nc.sync.dma_start(t_[:], a_dram.rearrange("p g t -> p (g t)"))
            nc.sync.dma_start(dbg["a"].rearrange("p g t -> p (g t)"), t_[:])

    # ================= Phase D: LSTM recurrence =================
    if skip_lstm:
        NI_LSTM = 1
    with tc.tile_pool(name="lw", bufs=1) as lwp, \
         tc.tile_pool(name="lst", bufs=1) as lsp, \
         tc.tile_pool(name="lio", bufs=2) as liop, \
         tc.tile_pool(name="ltmp", bufs=2) as ltp, \
         tc.tile_pool(name="lpsum", bufs=2, space="PSUM") as lpp:
        whh_sb = lwp.tile([128, KH, 2048], dt.float32, tag="whh")
        nc.sync.dma_start(whh_sb[:], whh.rearrange("p (k g) -> p k g", k=KH))
        h_state = lsp.tile([128, KH], dt.float32, tag="hstate")
        c_state = lsp.tile([128, KH], dt.float32, tag="cstate")
        h0c0_sb = lsp.tile([128, 2 * KH], dt.float32, tag="h0c0")
        nc.sync.dma_start(h0c0_sb[:], h0c0)
        nc.vector.tensor_copy(h_state[:], h0c0_sb[:, 0:KH])
        nc.vector.tensor_copy(c_state[:], h0c0_sb[:, KH:2 * KH])

        with tc.For_i(0, NI_LSTM, 1, hint_engines=(mybir.EngineType.PE,),
                      staggered_reset=stag) as ci:
            a_sb = liop.tile([128, NG, U], dt.float32, tag="a_sb")
            nc.sync.dma_start(a_sb[:], a_dram[:, :, bass.ts(ci, U)])
            # one h tile per hidden group so group deps stay fine-grained
            hsv = [liop.tile([128, U], dt.float32, tag=f"hs{g}", name=f"hs{g}")
                   for g in range(KH)]
            for u in range(U):
                gp_ = lpp.tile([128, NG], dt.float32, tag="gates")
                for g in range(KH):
                    for ty in range(4):
                        col = 4 * g + ty
                        import os as _os
                        _kh = 1 if _os.environ.get("K_PROBE") else KH
                        for k in range(_kh):
                            rhs = (h_state[:, k:k + 1] if u == 0
                                   else hsv[k][:, u - 1:u])
                            nc.tensor.matmul(
                                gp_[:, col:col + 1],
                                lhsT=whh_sb[:, k, 128 * col:128 * (col + 1)],
                                rhs=rhs, start=(k == 0), stop=(k == _kh - 1))
                    # epilogue for group g (cols 4g..4g+3 = i,f,o,g~)
                    gs = ltp.tile([128, 4], dt.float32, tag="gs")
                    nc.vector.tensor_add(gs[:], gp_[:, 4 * g:4 * g + 4],
                                         a_sb[:, 4 * g:4 * g + 4, u])
                    nc.scalar.activation(gs[:, 0:3], gs[:, 0:3], AF.Sigmoid)
                    nc.scalar.activation(gs[:, 3:4], gs[:, 3:4], AF.Tanh)
                    t1 = ltp.tile([128, 1], dt.float32, tag="t1")
                    nc.vector.tensor_mul(t1[:], gs[:, 1:2], c_state[:, g:g + 1])
                    t2 = ltp.tile([128, 1], dt.float32, tag="t2")
                    nc.vector.tensor_mul(t2[:], gs[:, 0:1], gs[:, 3:4])
                    nc.vector.tensor_add(c_state[:, g:g + 1], t1[:], t2[:])
                    tc_ = ltp.tile([128, 1], dt.float32, tag="tc")
                    nc.scalar.activation(tc_[:], c_state[:, g:g + 1], AF.Tanh)
                    nc.vector.tensor_mul(hsv[g][:, u:u + 1], gs[:, 2:3], tc_[:])
            for g in range(KH):
                nc.vector.tensor_copy(h_state[:, g:g + 1], hsv[g][:, U - 1:U])
                nc.sync.dma_start(h_dram[:, g, bass.ts(ci, U)], hsv[g][:])
    if debug:
        with tc.tile_pool(name="dbgh", bufs=2) as dp:
            t_ = dp.tile([128, KH * L], dt.float32, tag="dbgh")
            nc.sync.dma_start(t_[:], h_dram.rearrange("p k t -> p (k t)"))
            nc.sync.dma_start(dbg["hf"].rearrange("p k t -> p (k t)"), t_[:])

    # ================= Phase E: emission half =================
    emis_sb = pers.tile([128, L], dt.float32, tag="emis")
    with tc.tile_pool(name="ew", bufs=1) as ewp, \
         tc.tile_pool(name="eio", bufs=2) as eiop, \
         tc.tile_pool(name="epsum", bufs=2, space="PSUM") as epp:
        lin_sb = ewp.tile([128, KH, 128], dt.float32, tag="lin")
        nc.sync.dma_start(lin_sb[:], linT.rearrange("p (k m) -> p k m", k=KH))
        linb_sb = ewp.tile([128, 1], dt.float32, tag="linb")
        nc.sync.dma_start(linb_sb[:], linb)
        for ch in range(L // NCHX):
            hch = eiop.tile([128, KH, NCHX], dt.float32, tag="hch")
            nc.sync.dma_start(hch[:], h_dram[:, :, NCHX * ch:NCHX * (ch + 1)])
            ps = epp.tile([128, NCHX], dt.float32, tag="ep")
            for k in range(KH):
                nc.tensor.matmul(ps[:], lhsT=lin_sb[:, k, :],
                                 rhs=hch[:, k, :],
                                 start=(k == 0), stop=(k == KH - 1))
            nc.vector.tensor_scalar_add(
                emis_sb[:, NCHX * ch:NCHX * (ch + 1)], ps[:], linb_sb[:, 0:1])

    # ================= Phase F: exchange (reverse via perm + AllReduce) ====
    with tc.tile_pool(name="fx", bufs=2) as fxp, \
         tc.tile_pool(name="fpsum", bufs=2, space="PSUM") as fpp, \
         tc.tile_pool(name="fdram", bufs=1, space="DRAM") as fdp:
        tm_sb = fxp.tile([128, NT, 128], dt.float32, tag="tm")
        for c in range(NT):
            pt = fpp.tile([128, 128], dt.float32, tag="fpt")
            nc.tensor.transpose(pt[:], emis_sb[:, 128 * c:128 * (c + 1)], ident_sb[:])
            nc.vector.tensor_copy(tm_sb[:, c, :], pt[:])
        nc.sync.dma_start(tm_dram.rearrange("(c p) f -> p c f", p=128), tm_sb[:])
        perm_sb = fxp.tile([128, NT], dt.int32, tag="perm")
        nc.sync.dma_start(perm_sb[:], perm.rearrange("(c p) -> p c", p=128))
        g_sb = fxp.tile([128, NT, 128], dt.float32, tag="g_sb")
        for c in range(NT):
            nc.gpsimd.indirect_dma_start(
                out=g_sb[:, c, :], out_offset=None, in_=tm_dram,
                in_offset=bass.IndirectOffsetOnAxis(ap=perm_sb[:, c:c + 1], axis=0))
        contrib = fdp.tile([L, 128], dt.float32, tag="contrib")
        enat = fdp.tile([L, 128], dt.float32, tag="enat")
        nc.sync.dma_start(contrib[:].rearrange("(c p) f -> p c f", p=128), g_sb[:])
        nc.gpsimd.collective_compute(
            "AllReduce", mybir.AluOpType.add, replica_groups=groups,
            ins=[contrib[:]], outs=[enat[:]])
        g2_sb = fxp.tile([128, NT, 128], dt.float32, tag="g2")
        nc.sync.dma_start(g2_sb[:], enat[:].rearrange("(c p) f -> p c f", p=128))
        for c in range(NT):
            pt = fpp.tile([128, 128], dt.float32, tag="fpt")
            nc.tensor.transpose(pt[:], g2_sb[:, c, :], ident_sb[:])
            nc.vector.tensor_copy(emis_sb[:, 128 * c:128 * (c + 1)], pt[:])
        nc.sync.dma_start(e_dram[:], emis_sb[:])
    if debug:
        with tc.tile_pool(name="dbge", bufs=2) as dp:
            nc.sync.dma_start(dbg["emis"], emis_sb[:])

    # ================= Phase G: CRF forward scan =================
    with tc.tile_pool(name="cw", bufs=1) as cwp, \
         tc.tile_pool(name="cio", bufs=2) as ciop, \
         tc.tile_pool(name="ctmp", bufs=3) as ctp, \
         tc.tile_pool(name="cpsum", bufs=2, space="PSUM") as cpp:
        trans_sb = cwp.tile([128, 128], dt.float32, tag="trans")
        nc.sync.dma_start(trans_sb[:], transA)
        s_state = cwp.tile([128, 1], dt.float32, tag="sstate")  # pre-emission S
        initv_sb = cwp.tile([128, 1], dt.float32, tag="initv")
        nc.sync.dma_start(initv_sb[:], initv)
        nc.vector.tensor_copy(s_state[:], initv_sb[:])

        if skip_crf:
            NC_CRF = 1
        with tc.For_i(0, NC_CRF, 1,
                      hint_engines=(mybir.EngineType.PE, mybir.EngineType.DVE),
                      staggered_reset=stag) as ci:
            ech = ciop.tile([128, CU], dt.float32, tag="ech")
            nc.sync.dma_start(ech[:], e_dram[:, bass.ts(ci, CU)])
            hch = ciop.tile([128, CU], dt.uint32, tag="hch")
            m8p = None
            for u in range(CU):
                # step t = ci*CU+u+1: tmp[i,j] = (trans[i,j] + S_{t-1}[i]) + e_{t-1}[i]
                tmp = ctp.tile([128, 128], dt.float32, tag="ctmp")
                s_ap = s_state[:, 0:1] if u == 0 else m8p[:, 0:1]
                nc.vector.tensor_scalar(
                    tmp[:], trans_sb[:], s_ap, ech[:, u:u + 1],
                    op0=OP.add, op1=OP.add)
                pt = cpp.tile([128, 128], dt.float32, tag="cpt")
                nc.tensor.transpose(pt[:], tmp[:], ident_sb[:])
                m8 = ctp.tile([128, 8], dt.float32, tag="m8")
                nc.vector.max(out=m8[:], in_=pt[:])
                h8 = ctp.tile([128, 8], dt.uint32, tag="h8")
                nc.vector.max_index(out=h8[:], in_max=m8[:], in_values=pt[:])
                nc.vector.tensor_copy(hch[:, u:u + 1], h8[:, 0:1])
                m8p = m8
            nc.vector.tensor_copy(s_state[:], m8p[:, 0:1])
            nc.sync.dma_start(hist_out[:, bass.ts(ci, CU)], hch[:])

        # scorevec = (S_last + e_{L-1}) + endv
        endv_sb = cwp.tile([128, 1], dt.float32, tag="endv")
        nc.sync.dma_start(endv_sb[:], endv)
        sc = cwp.tile([128, 1], dt.float32, tag="sc")
        nc.vector.tensor_add(sc[:], s_state[:], emis_sb[:, L - 1:L])
        nc.vector.tensor_add(sc[:], sc[:], endv_sb[:])
        nc.sync.dma_start(score_out, sc[:])

    ctx.close()
    return dbg


def backtrace(hist, scorevec, L):
    """hist: [128, L-1] (col s = ref hist[s]); scorevec: [128]."""
    last = int(np.argmax(scorevec))
    best = np.float32(scorevec[last])
    path = np.empty(L, np.int32)
    path[L - 1] = last
    cur = last
    for t in range(L - 2, -1, -1):
        cur = int(hist[cur, t])
        path[t] = cur
    return path, best


def kernel(**inputs):
    import concourse.bacc as bacc
    import concourse.tile as tile
    from concourse.bass_utils import run_bass_kernel_spmd

    inputs = {k: np.asarray(v) for k, v in inputs.items()}
    n_cores = 8
    nc = bacc.Bacc("TRN2", target_bir_lowering=False, debug=False,
                   num_devices=n_cores)
    with tile.TileContext(nc) as tc:
        build_program(nc, tc, n_cores, L=L_FULL, U=16)
    nc.compile()

    in_fwd = make_core_inputs(inputs, core_is_bwd=False)
    in_bwd = make_core_inputs(inputs, core_is_bwd=True)
    in_maps = [in_fwd if c % 2 == 0 else in_bwd for c in range(n_cores)]
    res = run_bass_kernel_spmd(nc, in_maps, list(range(n_cores)))
    r0 = res.results[0]
    path, best = backtrace(r0["hist"], r0["scorevec"].reshape(-1), L_FULL)
    return path, best
